# revision 5
# baseline (speedup 1.0000x reference)
"""ConvCapsuleLayer Trainium2 kernel v2.2: fp16 7-matmul conv + tree-add routing.

Sharding: 8 cores = batch(4) x H-halves(2). Per core: 4 input-capsule images,
64 output rows x 128 cols. Conv: per (img,row) 7 bf16 matmuls (5 kw-replicated
"repA" windows covering kw=0..3 + 2 quad windows covering kw=4). Routing in
16-row super-blocks, (a,co)-major bf16 layout so every big DVE op runs packed
2x; atom-reductions via 5-level tree adds (tensor_reduce has no fast mode);
squash scale via Newton rsqrt on GPSIMD (no Ln -> single ACT table set).
"""
import numpy as np
import ml_dtypes
from contextlib import ExitStack

import concourse.bass as bass
import concourse.tile as tile
from concourse import bacc, mybir
from concourse.bass_utils import run_bass_kernel_spmd

KK = 5
CI, CO, A = 4, 8, 32
COA = CO * A  # 256
NROWS = 64
RSB = 16  # rows per routing super-block
NSB = NROWS // RSB
G = 8  # rows per psum group

BF16 = mybir.dt.float16  # fp16: same speed as bf16, 4x the mantissa
F32 = mybir.dt.float32
I32 = mybir.dt.int32
MULT = mybir.AluOpType.mult
ADD = mybir.AluOpType.add
SUB = mybir.AluOpType.subtract
DIV = mybir.AluOpType.divide
SHR = mybir.AluOpType.logical_shift_right
AX = mybir.AxisListType.X
AF = mybir.ActivationFunctionType
BFNP = np.float16
RSQRT_MAGIC = 0x5F3759DF


def build_program():
    nc = bacc.Bacc("TRN2", target_bir_lowering=False, debug=False, num_devices=1)

    repa_d = nc.dram_tensor("repa", [CI, 128, 68 * 128], BF16, kind="ExternalInput").ap()
    quads_d = nc.dram_tensor("quads", [CI, 128, 17 * 132], BF16, kind="ExternalInput").ap()
    w_d = nc.dram_tensor("wstk", [128, 13 * COA], BF16, kind="ExternalInput").ap()
    bt_d = nc.dram_tensor("bt", [128, COA], BF16, kind="ExternalInput").ap()
    out_d = nc.dram_tensor("out", [NROWS, 128, COA], BF16, kind="ExternalOutput").ap()

    with tile.TileContext(nc) as tc, ExitStack() as ctx:
        cpool = ctx.enter_context(tc.tile_pool(name="const", bufs=1))
        rpool = ctx.enter_context(tc.tile_pool(name="repa", bufs=1))
        vpool = ctx.enter_context(tc.tile_pool(name="votes", bufs=1))
        mpool = ctx.enter_context(tc.tile_pool(name="mids", bufs=1))
        ppool = ctx.enter_context(tc.tile_pool(name="pres", bufs=1))
        spool = ctx.enter_context(tc.tile_pool(name="small", bufs=1))
        pspool = ctx.enter_context(tc.tile_pool(name="ps", bufs=1, space="PSUM"))

        quads = cpool.tile([128, CI * 17 * 132], BF16)
        for i in range(CI):
            nc.sync.dma_start(quads[:, i * 17 * 132 : (i + 1) * 17 * 132], quads_d[i])
        wstk = cpool.tile([128, 13 * COA], BF16)
        nc.sync.dma_start(wstk[:], w_d[:])
        btile = cpool.tile([128, COA], BF16)
        nc.sync.dma_start(btile[:], bt_d[:])

        SBS = [4, 12, 16, 16, 16]  # super-block row counts (sum 64)
        SB0 = [sum(SBS[:k]) for k in range(len(SBS))]

        def bt_bc(rsb):
            return btile[:].unsqueeze(1).broadcast_to((128, rsb, COA))

        def dma_repa(k):
            r0, rsb = SB0[k], SBS[k]
            ts = []
            for i in range(CI):
                t = rpool.tile([128, 20 * 128], BF16, tag="ra", bufs=2, name=f"ra{k}_{i}")
                nc.sync.dma_start(
                    t[:, 0 : (rsb + 4) * 128],
                    repa_d[i][:, r0 * 128 : (r0 + rsb + 4) * 128],
                )
                ts.append(t)
            return ts

        def conv_img(k, rts, V4, i):
            r0, rsb = SB0[k], SBS[k]
            for g, g0 in enumerate(range(0, rsb, G)):
                gn = min(G, rsb - g0)
                ps = pspool.tile([128, G * COA], F32, tag="ps", bufs=2, name=f"ps{k}_{i}_{g}")
                for rl in range(gn):
                    r = r0 + g0 + rl
                    lr = g0 + rl
                    od = ps[:, rl * COA : (rl + 1) * COA]
                    for h in range(KK):
                        nc.tensor.matmul(
                            od,
                            rts[i][:, (lr + h) * 128 : (lr + h + 1) * 128],
                            wstk[:, h * COA : (h + 1) * COA],
                            start=(h == 0),
                            stop=False,
                            tile_position=(0, 0),
                            skip_group_check=True,
                        )
                    q0, m = r // 4, r % 4
                    for sgn in range(2):
                        nc.tensor.matmul(
                            od,
                            quads[:, (i * 17 + q0 + sgn) * 132 + 4 : (i * 17 + q0 + sgn) * 132 + 132],
                            wstk[:, (5 + m * 2 + sgn) * COA : (5 + m * 2 + sgn + 1) * COA],
                            start=False,
                            stop=(sgn == 1),
                            tile_position=(0, 0),
                            skip_group_check=True,
                        )
                nc.scalar.copy(
                    V4[:, g0 : g0 + gn, i, :],
                    ps[:, 0 : gn * COA].rearrange("p (r x) -> p r x", r=gn),
                )

        def split_tt(out4, in04, in14, op, rsb):
            """Row-split tensor_tensor: DVE rows 0..rd, GPSIMD rows rd..rsb.
            Small ops stay whole on DVE (split overhead beats the gain)."""
            if out4.free_size() < 1024:
                nc.vector.tensor_tensor(out4, in04, in14, op)
                return
            rd = (rsb * 13 + 15) // 16
            nc.vector.tensor_tensor(out4[:, 0:rd], in04[:, 0:rd], in14[:, 0:rd], op)
            if rd < rsb:
                nc.gpsimd.tensor_tensor(out4[:, rd:rsb], in04[:, rd:rsb], in14[:, rd:rsb], op)

        def tree_a(src, dst, nm, rsb):
            """Sum over a: src [128, RSB*A*CO] fp16 (a,co-major) -> dst [128,rsb*CO] f32."""
            cur, w = src, A
            for wn in (16, 8, 4, 2):
                t = spool.tile([128, RSB * wn * CO], BF16, tag=f"t{wn}", bufs=1, name=f"t{wn}_{nm}")
                c4 = cur[:].rearrange("p (r a co) -> p r a co", r=RSB, a=2 * wn)
                o4 = t[:].rearrange("p (r a co) -> p r a co", r=RSB, a=wn)
                split_tt(o4[:, 0:rsb], c4[:, 0:rsb, 0:wn, :], c4[:, 0:rsb, wn : 2 * wn, :], ADD, rsb)
                cur = t
            c4 = cur[:].rearrange("p (r a co) -> p r a co", r=RSB, a=2)
            split_tt(dst, c4[:, 0:rsb, 0, :], c4[:, 0:rsb, 1, :], ADD, rsb)

        def squash(pre, nm, expand, rsb):
            """s = sqrt(n2)/(1+n2), Newton rsqrt on DVE; fp16 [128,rsb*CO]."""
            nco = rsb * CO
            sq = mpool.tile([128, RSB * COA], BF16, tag="ma", bufs=2, name="sq" + nm)
            nc.scalar.activation(sq[:, 0 : rsb * COA], pre[:, 0 : rsb * COA], AF.Square)
            n2 = spool.tile([128, RSB * CO], F32, tag="n2", bufs=2, name="n2" + nm)
            tree_a(sq, n2[:, 0:nco].rearrange("p (r co) -> p r co", r=rsb), nm, rsb)
            sh = spool.tile([128, RSB * CO], I32, tag="sh", name="sh" + nm)
            nc.vector.tensor_scalar(sh[:, 0:nco], n2[:, 0:nco].bitcast(I32), 1, None, SHR)
            y = spool.tile([128, RSB * CO], F32, tag="y", bufs=2, name="y" + nm)
            nc.vector.tensor_scalar(y[:, 0:nco].bitcast(I32), sh[:, 0:nco], -1, RSQRT_MAGIC, MULT, ADD)
            for it in range(1):
                t1 = spool.tile([128, RSB * CO], F32, tag="nr1", bufs=1, name=f"nr1{nm}_{it}")
                nc.gpsimd.tensor_tensor(t1[:, 0:nco], y[:, 0:nco], y[:, 0:nco], MULT)
                nc.gpsimd.tensor_tensor(t1[:, 0:nco], t1[:, 0:nco], n2[:, 0:nco], MULT)
                nc.vector.tensor_scalar(t1[:, 0:nco], t1[:, 0:nco], -0.5, 1.5, MULT, ADD)
                y2 = spool.tile([128, RSB * CO], F32, tag="y", bufs=2, name=f"y{nm}_{it}")
                nc.gpsimd.tensor_tensor(y2[:, 0:nco], y[:, 0:nco], t1[:, 0:nco], MULT)
                y = y2
            n_ = spool.tile([128, RSB * CO], F32, tag="nn", name="nn" + nm)
            nc.gpsimd.tensor_tensor(n_[:, 0:nco], n2[:, 0:nco], y[:, 0:nco], MULT)
            d = spool.tile([128, RSB * CO], F32, tag="d", name="d" + nm)
            nc.vector.tensor_scalar(d[:, 0:nco], n2[:, 0:nco], 1.0, None, ADD)
            di = spool.tile([128, RSB * CO], F32, tag="di", name="di" + nm)
            nc.vector.reciprocal(di[:, 0:nco], d[:, 0:nco])
            s = spool.tile([128, RSB * CO], BF16, tag="s", bufs=2, name="s" + nm)
            nc.vector.tensor_tensor(s[:, 0:nco], n_[:, 0:nco], di[:, 0:nco], MULT)
            if expand:
                sx = ppool.tile([128, RSB * COA], BF16, tag="sx", name="sx" + nm)
                hh = max(1, rsb // 2)
                for lo, hi in ((0, hh), (hh, rsb)):
                    if hi > lo:
                        nc.scalar.copy(
                            sx[:, lo * COA : hi * COA].rearrange("p (r a co) -> p r a co", r=hi - lo, a=A),
                            s[:, lo * CO : hi * CO]
                            .rearrange("p (r co) -> p r co", r=hi - lo)
                            .unsqueeze(2)
                            .broadcast_to((128, hi - lo, A, CO)),
                        )
                return sx
            return s

        def agreement(V4, pre, s, tg, nm, rsb):
            """lc = s * sum_a(V*pre) -> [128, rsb*CI*CO] f32."""
            lraw = spool.tile([128, RSB * CI * CO], F32, tag="lraw", name="lraw" + nm)
            lraw4 = lraw[:].rearrange("p (r ci co) -> p r ci co", r=RSB, ci=CI)
            p3 = pre[:].rearrange("p (r x) -> p r x", r=RSB)
            for ci in range(CI):
                m = mpool.tile([128, RSB * COA], BF16, tag="ma", bufs=2, name=f"mag{nm}_{ci}")
                split_tt(
                    m[:].rearrange("p (r x) -> p r x", r=RSB)[:, 0:rsb],
                    V4[:, 0:rsb, ci, :],
                    p3[:, 0:rsb],
                    MULT,
                    rsb,
                )
                tree_a(m, lraw4[:, 0:rsb, ci, :], f"{nm}c{ci}", rsb)
            lc = spool.tile([128, RSB * CI * CO], F32, tag=tg, name="lc" + nm)
            s_b = (
                s[:, 0 : rsb * CO]
                .rearrange("p (r co) -> p r co", r=rsb)
                .unsqueeze(2)
                .broadcast_to((128, rsb, CI, CO))
            )
            nc.vector.tensor_tensor(
                lc[:, 0 : rsb * CI * CO].rearrange("p (r ci co) -> p r ci co", r=rsb, ci=CI),
                lraw4[:, 0:rsb],
                s_b,
                MULT,
            )
            return lc

        def softmax(l, nm, rsb):
            """rt = e / sum_co(e), e = exp(l - max_co(l)) -> [128, rsb*CI*CO] fp16."""
            nrc = rsb * CI
            curm, wm = l, CO
            while wm > 1:
                wm //= 2
                tm = spool.tile([128, RSB * CI * wm], F32, tag=f"mx{wm}", name=f"mx{wm}_{nm}")
                cm = curm[:].rearrange("p (rc co) -> p rc co", co=2 * wm)
                nc.vector.tensor_tensor(
                    tm[:].rearrange("p (rc co) -> p rc co", co=wm)[:, 0:nrc],
                    cm[:, 0:nrc, 0:wm],
                    cm[:, 0:nrc, wm : 2 * wm],
                    mybir.AluOpType.max,
                )
                curm = tm
            ld = spool.tile([128, RSB * CI * CO], F32, tag="ld", name="ld" + nm)
            nc.vector.tensor_tensor(
                ld[:].rearrange("p (rc co) -> p rc co", co=CO)[:, 0:nrc],
                l[:].rearrange("p (rc co) -> p rc co", co=CO)[:, 0:nrc],
                curm[:].rearrange("p (rc x) -> p rc x", x=1)[:, 0:nrc].broadcast_to((128, nrc, CO)),
                SUB,
            )
            e = spool.tile([128, RSB * CI * CO], BF16, tag="e", bufs=1, name="e" + nm)
            nc.scalar.activation(e[:, 0 : nrc * CO], ld[:, 0 : nrc * CO], AF.Exp)
            cur, w = e, CO
            while w > 2:
                w //= 2
                t = spool.tile([128, RSB * CI * w], BF16, tag=f"zc{w}", bufs=1, name=f"zc{w}_{nm}")
                c3 = cur[:].rearrange("p (rc co) -> p rc co", co=2 * w)
                nc.vector.tensor_tensor(
                    t[:].rearrange("p (rc co) -> p rc co", co=w)[:, 0:nrc],
                    c3[:, 0:nrc, 0:w],
                    c3[:, 0:nrc, w : 2 * w],
                    ADD,
                )
                cur = t
            Z = spool.tile([128, RSB * CI], F32, tag="Z", name="Z" + nm)
            c3 = cur[:].rearrange("p (rc co) -> p rc co", co=2)
            nc.vector.tensor_tensor(
                Z[:].rearrange("p (rc x) -> p rc x", x=1)[:, 0:nrc],
                c3[:, 0:nrc, 0:1],
                c3[:, 0:nrc, 1:2],
                ADD,
            )
            zi = spool.tile([128, RSB * CI], F32, tag="zi", name="zi" + nm)
            nc.vector.reciprocal(zi[:, 0:nrc], Z[:, 0:nrc])
            rt = spool.tile([128, RSB * CI * CO], BF16, tag="rt", bufs=1, name="rt" + nm)
            nc.vector.tensor_tensor(
                rt[:].rearrange("p (rc co) -> p rc co", co=CO)[:, 0:nrc],
                e[:].rearrange("p (rc co) -> p rc co", co=CO)[:, 0:nrc],
                zi[:].rearrange("p (rc x) -> p rc x", x=1)[:, 0:nrc].broadcast_to((128, nrc, CO)),
                MULT,
            )
            return rt

        def weighted(V4, rt, nm, rsb):
            """pre = sum_ci rt*V + b -> [128, rsb*COA] fp16 ((a,co)-major => packed)."""
            rt4 = rt[:].rearrange("p (r ci co) -> p r ci co", r=RSB, ci=CI)
            ms = []

            def r3(t):
                return t[:].rearrange("p (r x) -> p r x", r=RSB)[:, 0:rsb]

            for ci in range(CI):
                m = mpool.tile([128, RSB * COA], BF16, tag="ma", bufs=2, name=f"mw{nm}_{ci}")
                split_tt(
                    m[:].rearrange("p (r a co) -> p r a co", r=RSB, a=A)[:, 0:rsb],
                    V4[:, 0:rsb, ci, :].rearrange("p r (a co) -> p r a co", a=A),
                    rt4[:, 0:rsb, ci, :].unsqueeze(2).broadcast_to((128, rsb, A, CO)),
                    MULT,
                    rsb,
                )
                ms.append(m)
                if ci == 1:
                    acc1 = mpool.tile([128, RSB * COA], BF16, tag="acc", bufs=2, name=f"acc1{nm}")
                    split_tt(r3(acc1), r3(ms[0]), r3(ms[1]), ADD, rsb)
            acc2 = mpool.tile([128, RSB * COA], BF16, tag="acc", bufs=2, name=f"acc2{nm}")
            split_tt(r3(acc2), r3(ms[2]), r3(ms[3]), ADD, rsb)
            acc3 = mpool.tile([128, RSB * COA], BF16, tag="acc3", name=f"acc3{nm}")
            split_tt(r3(acc3), r3(acc1), r3(acc2), ADD, rsb)
            pre = ppool.tile([128, RSB * COA], BF16, tag="pre", bufs=2, name=f"pre{nm}")
            split_tt(r3(pre), r3(acc3), bt_bc(rsb), ADD, rsb)
            return pre

        def routing_sb(k, V4, h1, h2):
            rsb = SBS[k]

            def r3(t):
                return t[:].rearrange("p (r x) -> p r x", r=RSB)[:, 0:rsb]

            h12 = mpool.tile([128, RSB * COA], BF16, tag="acc3", name=f"h12_{k}")
            split_tt(r3(h12), r3(h1), r3(h2), ADD, rsb)
            # scalar_tensor_tensor has no fast DVE mode (1x); ts-scale (4x) + split add
            h8 = mpool.tile([128, RSB * COA], BF16, tag="ma", bufs=2, name=f"h8_{k}")
            nc.vector.tensor_scalar(
                h8[:, 0 : rsb * COA], h12[:, 0 : rsb * COA], 0.125, None, MULT
            )
            pre1 = ppool.tile([128, RSB * COA], BF16, tag="pre", bufs=2, name=f"pre1_{k}")
            split_tt(r3(pre1), r3(h8), bt_bc(rsb), ADD, rsb)
            s1 = squash(pre1, f"1_{k}", False, rsb)
            l1 = agreement(V4, pre1, s1, "lc1", f"1_{k}", rsb)

            rt2 = softmax(l1, f"2_{k}", rsb)
            pre2 = weighted(V4, rt2, f"2_{k}", rsb)
            s2 = squash(pre2, f"2_{k}", False, rsb)
            lc2 = agreement(V4, pre2, s2, "lc2", f"2_{k}", rsb)
            l2 = spool.tile([128, RSB * CI * CO], F32, tag="l2", name=f"l2_{k}")
            nc.vector.tensor_tensor(
                l2[:, 0 : rsb * CI * CO], l1[:, 0 : rsb * CI * CO], lc2[:, 0 : rsb * CI * CO], ADD
            )

            rt3 = softmax(l2, f"3_{k}", rsb)
            pre3 = weighted(V4, rt3, f"3_{k}", rsb)
            sx = squash(pre3, f"3_{k}", True, rsb)
            act = ppool.tile([128, RSB * COA], BF16, tag="act", bufs=2, name=f"act_{k}")
            split_tt(r3(act), r3(pre3), r3(sx), MULT, rsb)
            r0 = SB0[k]
            nc.sync.dma_start(
                out_d[r0 : r0 + rsb].rearrange("r p x -> p r x"),
                act[:, 0 : rsb * COA].rearrange("p (r x) -> p r x", r=rsb),
            )

        rts = dma_repa(0)
        for k in range(len(SBS)):
            rsb = SBS[k]
            V = vpool.tile([128, RSB * CI * COA], BF16, tag="V", bufs=2, name=f"V{k}")
            V4 = V[:].rearrange("p (r ci x) -> p r ci x", r=RSB, ci=CI)
            # conv imgs 0,1 then start the vote-sum halves early
            conv_img(k, rts, V4, 0)
            conv_img(k, rts, V4, 1)
            h1 = mpool.tile([128, RSB * COA], BF16, tag="ma", bufs=2, name=f"h1_{k}")
            split_tt(
                h1[:].rearrange("p (r x) -> p r x", r=RSB)[:, 0:rsb],
                V4[:, 0:rsb, 0, :], V4[:, 0:rsb, 1, :], ADD, rsb,
            )
            conv_img(k, rts, V4, 2)
            conv_img(k, rts, V4, 3)
            h2 = mpool.tile([128, RSB * COA], BF16, tag="ma", bufs=2, name=f"h2_{k}")
            split_tt(
                h2[:].rearrange("p (r x) -> p r x", r=RSB)[:, 0:rsb],
                V4[:, 0:rsb, 2, :], V4[:, 0:rsb, 3, :], ADD, rsb,
            )
            if k + 1 < len(SBS):
                rts = dma_repa(k + 1)
            routing_sb(k, V4, h1, h2)

    nc.compile()
    return nc


_NC_CACHE = None


def _get_nc():
    global _NC_CACHE
    if _NC_CACHE is None:
        _NC_CACHE = build_program()
    return _NC_CACHE


def host_prep(input_tensor, W, b):
    x = np.asarray(input_tensor, np.float32)
    W = np.asarray(W, np.float32)
    b = np.asarray(b, np.float32)
    B, H, Wd, Ci, Ai = x.shape

    xp = np.zeros((B, H + 4, Wd + 4, Ci, Ai), np.float32)
    xp[:, 2 : H + 2, 2 : Wd + 2] = x

    # reorder output channels to (a_out, co)-major
    Wa = W.reshape(KK, KK, Ai, CO, A).transpose(0, 1, 2, 4, 3).reshape(KK, KK, Ai, COA)

    wstk = np.zeros((128, 13, COA), np.float32)
    wstk[:, 0:KK, :] = Wa[:, 0:4].transpose(1, 2, 0, 3).reshape(128, KK, COA)
    for m in range(4):
        for j in range(4):
            kh = j - m
            if 0 <= kh <= 4:
                wstk[j * 32 : (j + 1) * 32, 5 + m * 2 + 0, :] = Wa[kh, 4]
            kh = 4 + j - m
            if 0 <= kh <= 4:
                wstk[j * 32 : (j + 1) * 32, 5 + m * 2 + 1, :] = Wa[kh, 4]
    wstk = wstk.reshape(128, 13 * COA).astype(BFNP)

    ba = b.reshape(CO, A).T.reshape(COA)  # (a, co)-major
    btile = np.broadcast_to(ba.reshape(1, COA), (128, COA)).astype(BFNP)

    in_maps = []
    for bb in range(B):
        for hh in range(2):
            shard = xp[bb, hh * 64 : hh * 64 + 68]  # [68, 132, Ci, Ai]
            img = np.ascontiguousarray(shard.transpose(2, 0, 3, 1))  # [Ci,68,Ai,132]
            repa = np.stack(
                [img[:, :, :, kw : kw + 128] for kw in range(4)], axis=1
            )
            repa = repa.transpose(0, 1, 3, 2, 4).reshape(CI, 128, 68 * 128)
            qd = img.reshape(CI, 17, 4, Ai, 132).transpose(0, 2, 3, 1, 4)
            qd = qd.reshape(CI, 128, 17 * 132)
            in_maps.append(
                {
                    "repa": repa.astype(BFNP),
                    "quads": qd.astype(BFNP),
                    "wstk": wstk,
                    "bt": btile,
                }
            )
    return in_maps


def kernel(input_tensor, W, b):
    x = np.asarray(input_tensor, np.float32)
    B, H, Wd, Ci, Ai = x.shape
    in_maps = host_prep(x, W, b)
    nc = _get_nc()
    res = run_bass_kernel_spmd(nc, in_maps, core_ids=list(range(8)))
    out = np.zeros((B, H, Wd, CO, A), np.float32)
    k = 0
    for bb in range(B):
        for hh in range(2):
            o = np.asarray(res.results[k]["out"]).astype(np.float32)
            # stored (a, co)-major -> (co, a)
            out[bb, hh * 64 : hh * 64 + 64] = (
                o.reshape(NROWS, 128, A, CO).transpose(0, 1, 3, 2)
            )
            k += 1
    return out
